# revision 80
# baseline (speedup 1.0000x reference)
"""Single-head causal attention (B=8, S=2048, D=1024, dk=64) on 8 trn2 cores.

Sharding: data-parallel over batch — one batch element per NeuronCore, no
collectives. Each core computes, for its batch b:
    q = x@Wq + bq; k = x@Wk + bk; v = x@Wv + bv
    out = softmax(causal(q k^T / 8)) @ v

Per-core kernel (bf16 matmuls for scores/AV — enables Fast Weight Load and
stays ~10x under the 2e-2 error gate):
  - x blocks are loaded in DESCENDING order across the sync+scalar HWDGE
    queues. Scores are computed TRANSPOSED per k-block stripe j
    (S^T[j] = kT_j.T @ qT over q-columns j*128..2048): stripe j only needs
    x blocks >= j, so score stripes run while early x blocks are still in
    flight, keeping the PE busy through the DMA-bound start.
  - exp on ACT writes P^T directly to SBUF in bf16 — the per-q-block P
    transposes + PSUM round-trips of the natural orientation do not exist.
  - Softmax denominators come from a ones-column matmul that reuses the
    P^T_j stationary weights during A@V accumulation. Max-subtraction is
    skipped (|s|/8 <= ~2 here, far from overflow).
  - bv is folded into v at PSUM evacuation (softmax rows sum to exactly 1,
    so A@(v+bv) = A@v + bv).
  - A@V blocks run in descending i so the small ones land last (short tail).
  - A PE warmup spin defeats the HAM clock gate (idle PE is throttled to
    1.2GHz; ~3.4us of sustained activity releases it to 2.4GHz).
"""

from contextlib import ExitStack

import numpy as np

S = 2048
D = 1024
DK = 64
B = 8
P = 128
NSB = S // P  # 16 seq blocks
KD = D // P  # 8 d_model chunks
NEG = -1.0e30
SCALE = 0.125  # 1/sqrt(dk)

_CACHE = {}


def _build():
    import concourse.bacc as bacc
    import concourse.mybir as mybir
    import concourse.tile as tile

    F32 = mybir.dt.float32
    F32R = mybir.dt.float32r
    BF16 = mybir.dt.bfloat16
    ACT = mybir.ActivationFunctionType

    nc = bacc.Bacc("TRN2", target_bir_lowering=False)
    # x/ident are declared float32r (same bytes as f32) so HWDGE loads them
    # without a cast and the PE transposes run at full fp32r rate
    x_d = nc.dram_tensor("x", [S, D], F32R, kind="ExternalInput")
    wq_d = nc.dram_tensor("wq", [D, DK], F32, kind="ExternalInput")
    bq_d = nc.dram_tensor("bq", [DK], F32, kind="ExternalInput")
    wk_d = nc.dram_tensor("wk", [D, DK], F32, kind="ExternalInput")
    bk_d = nc.dram_tensor("bk", [DK], F32, kind="ExternalInput")
    wv_d = nc.dram_tensor("wv", [D, D], F32, kind="ExternalInput")
    bv_d = nc.dram_tensor("bvbc", [P, D], F32, kind="ExternalInput")
    id_d = nc.dram_tensor("ident", [P, P], F32, kind="ExternalInput")
    mask_d = nc.dram_tensor("maskt", [P, P], F32, kind="ExternalInput")
    o_d = nc.dram_tensor("o", [S, D], F32, kind="ExternalOutput")

    with tile.TileContext(nc) as tc, ExitStack() as ctx:
        persist = ctx.enter_context(tc.tile_pool(name="persist", bufs=1))

        v_sb = [
            persist.tile([P, D], BF16, name=f"v{s}", tag=f"v{s}") for s in range(NSB)
        ]
        # P^T stripes: stripe j holds exp-scores for k-block j, q-cols j*128..S
        pstr = [
            persist.tile([P, (NSB - j) * P], BF16, name=f"pt{j}", tag=f"pt{j}")
            for j in range(NSB)
        ]
        xT = persist.tile([P, KD, S], BF16, name="xT", tag="xT")
        qT = persist.tile([DK, S], BF16, name="qT", tag="qT")
        kT = persist.tile([DK, S], BF16, name="kT", tag="kT")
        ident = persist.tile([P, P], BF16, name="ident", tag="ident")
        maskT = persist.tile([P, P], F32, name="maskT", tag="maskT")
        ones = persist.tile([P, 1], BF16, name="ones", tag="ones")
        bq_sb = persist.tile([DK, 1], F32, name="bq_sb", tag="bq_sb")
        bk_sb = persist.tile([DK, 1], F32, name="bk_sb", tag="bk_sb")
        bv_bc = persist.tile([P, D], F32, name="bv_bc", tag="bv_bc")
        wq_sb = persist.tile([P, KD, DK], BF16, name="wq_sb", tag="wq_sb")
        wk_sb = persist.tile([P, KD, DK], BF16, name="wk_sb", tag="wk_sb")
        wv_sb = persist.tile([P, KD, D], BF16, name="wv_sb", tag="wv_sb")
        warm = persist.tile([P, 512], BF16, name="warm", tag="warm")

        xin = ctx.enter_context(tc.tile_pool(name="xin", bufs=6))
        xbfp = ctx.enter_context(tc.tile_pool(name="xbfp", bufs=5))
        opool = ctx.enter_context(tc.tile_pool(name="opool", bufs=3))
        stat = ctx.enter_context(tc.tile_pool(name="stat", bufs=4))
        # PSUM: 4 banks of 512-wide tiles (warmup/transposes/projections/
        # scores/l) + 2x2 banks of 1024-wide accumulators (v-proj / A@V)
        psA = ctx.enter_context(tc.tile_pool(name="psA", bufs=4, space="PSUM"))
        psB = ctx.enter_context(tc.tile_pool(name="psB", bufs=2, space="PSUM"))

        # ---- trigger-only prologue: weights on the gpsimd SWDGE queue
        # (casting f32->bf16 for wv), x blocks DESCENDING on sync+scalar.
        # Engine queues carry only triggers here so DMA saturates at once. ----
        # sync+scalar queues carry ONLY the 16 x blocks (descending) so they
        # saturate on x; every constant/weight rides the gpsimd queue
        xbs = [None] * NSB
        xbfs = [None] * NSB
        # each block's halves split across BOTH queues: blocks complete
        # uniformly every ~3.3us in exactly the (descending) consumption
        # order, instead of pairs every ~6.6us
        for b in range(NSB - 1, -1, -1):
            xb = xin.tile([P, D], F32R, name=f"x{b}", tag="x")
            nc.sync.dma_start(xb[:, 0:512], x_d.ap()[b * P : (b + 1) * P, 0:512])
            nc.scalar.dma_start(
                xb[:, 512:1024], x_d.ap()[b * P : (b + 1) * P, 512:1024]
            )
            xbs[b] = xb
        nc.gpsimd.dma_start(ident[:], id_d.ap())
        nc.gpsimd.dma_start(bq_sb[:], bq_d.ap()[:, None])
        nc.gpsimd.dma_start(bk_sb[:], bk_d.ap()[:, None])
        nc.gpsimd.dma_start(maskT[:], mask_d.ap())
        nc.gpsimd.dma_start(wq_sb[:], wq_d.ap().rearrange("(ko p) m -> p ko m", p=P))
        nc.gpsimd.dma_start(wk_sb[:], wk_d.ap().rearrange("(ko p) m -> p ko m", p=P))
        wv_ap = wv_d.ap().rearrange("(ko p) m -> p ko m", p=P)
        nc.gpsimd.dma_start(wv_sb[:, :, 0:512], wv_ap[:, :, 0:512])
        nc.gpsimd.dma_start(wv_sb[:, :, 512:1024], wv_ap[:, :, 512:1024])
        # bv last: only the (DVE) evacuation of v waits on it, not the PE
        nc.gpsimd.dma_start(bv_bc[:], bv_d.ap())
        nc.vector.memset(ones[:], 1.0)
        nc.vector.memset(warm[:], 0.0)

        # ---- PE warmup / HAM keep-alive ----
        def fill(n):
            # one PSUM slot per burst, n overwrites into it: no pool churn,
            # no cross-engine bank serialization, pure PE activity
            wps = psB.tile([P, D], F32, name="wps", tag="b")
            for w in range(n):
                nc.tensor.matmul(wps[:, :512], warm[:, :P], warm[:], start=True, stop=True)

        fill(28)

        def cast_block(b):
            # half-granular: the h0 cast waits only the sync-queue half, so
            # the block's first transposes start before the scalar half lands
            xc = xbfp.tile([P, D], BF16, name=f"xc{b}", tag="xc")
            for h in range(2):
                hs = slice(h * 512, (h + 1) * 512)
                nc.vector.tensor_copy(out=xc[:, hs], in_=xbs[b][:, hs])
            xbfs[b] = xc

        def transpose_block(b):
            xc = xbfs[b]
            for h in range(2):
                pst = psA.tile([P, 512], BF16, name=f"pst{b}_{h}", tag="a")
                for kk in range(4):
                    k = h * 4 + kk
                    nc.tensor.transpose(
                        pst[:, kk * P : (kk + 1) * P],
                        xc[:, k * P : (k + 1) * P],
                        ident[:],
                    )
                nc.vector.tensor_copy(
                    out=xT[:, h * 4 : (h + 1) * 4, b * P : (b + 1) * P],
                    in_=pst.rearrange("p (k s) -> p k s", k=4),
                )

        def qk_proj(g):
            gsl = slice(g * 512, (g + 1) * 512)
            for w_sb, b_sb, outT in ((wq_sb, bq_sb, qT), (wk_sb, bk_sb, kT)):
                pqk = psA.tile([P, 512], F32, name=f"pqk{g}", tag="a")
                for k in range(KD):
                    nc.tensor.matmul(
                        pqk[:DK, :],
                        w_sb[:, k, :],
                        xT[:, k, gsl],
                        start=(k == 0),
                        stop=(k == KD - 1),
                    )
                # bias-add on DVE keeps the scalar engine free for triggers/exp
                nc.vector.tensor_scalar_add(outT[:, gsl], pqk[:DK, :], b_sb[:])

        def v_proj(b):
            pv = psB.tile([P, D], F32, name=f"pv{b}", tag="b")
            for n in range(2):
                ns = slice(n * 512, (n + 1) * 512)
                for k in range(KD):
                    nc.tensor.matmul(
                        pv[:, ns],
                        xT[:, k, b * P : (b + 1) * P],
                        wv_sb[:, k, ns],
                        start=(k == 0),
                        stop=(k == KD - 1),
                    )
                # bv folded in here; softmax rows sum to 1 so this is exact
                nc.vector.tensor_add(out=v_sb[b][:, ns], in0=pv[:, ns], in1=bv_bc[:, ns])

        def stripe(j):
            wj = (NSB - j) * P
            nch = (wj + 511) // 512
            for c in range(nch):
                w = min(512, wj - c * 512)
                s_ps = psA.tile([P, 512], F32, name=f"s{j}_{c}", tag="a")
                nc.tensor.matmul(
                    s_ps[:, :w],
                    kT[:, j * P : (j + 1) * P],
                    qT[:, j * P + c * 512 : j * P + c * 512 + w],
                    start=True,
                    stop=True,
                )
                if c == 0:  # diagonal 128x128 block: causal mask (transposed)
                    nc.vector.tensor_add(
                        out=s_ps[:, :P], in0=s_ps[:, :P], in1=maskT[:]
                    )
                nc.scalar.activation(
                    pstr[j][:, c * 512 : c * 512 + w],
                    s_ps[:, :w],
                    ACT.Exp,
                    scale=SCALE,
                )

        def av(i):
            o_ps = psB.tile([P, D], F32, name=f"o{i}", tag="b")
            l_ps = psA.tile([P, 512], F32, name=f"l{i}", tag="a")
            for j in range(i + 1):
                pT = pstr[j][:, (i - j) * P : (i - j + 1) * P]
                st = j == 0
                sp = j == i
                nc.tensor.matmul(o_ps[:, 0:512], pT, v_sb[j][:, 0:512], start=st, stop=sp)
                nc.tensor.matmul(
                    o_ps[:, 512:1024], pT, v_sb[j][:, 512:1024], start=st, stop=sp
                )
                # softmax denominator: reuses the loaded P^T_j weights
                nc.tensor.matmul(l_ps[:, 0:1], pT, ones[:], start=st, stop=sp)
            rl = stat.tile([P, 1], F32, name=f"rl{i}", tag="rl")
            nc.vector.reciprocal(rl[:], l_ps[:, 0:1])
            out_sb = opool.tile([P, D], F32, name=f"out{i}", tag="out")
            nc.scalar.mul(out_sb[:, 0:512], o_ps[:, 0:512], rl[:])
            nc.vector.tensor_scalar_mul(out_sb[:, 512:1024], o_ps[:, 512:1024], rl[:])
            nc.sync.dma_start(o_d.ap()[i * P : (i + 1) * P, :], out_sb[:])

        # ---- schedule: everything in x-arrival (descending-block) order.
        # Group g's stripes need only x blocks >= 4g, so they run while
        # earlier blocks are still loading. v-projs for the previous group
        # slot in behind (wv lands on gpsimd at ~1/3 of the x window). ----
        # x pairs land every ~6.6us; small distributed fills keep the HAM
        # clock warm across the waits, and once wv has landed (~40us) the
        # v-projections become the filler between transpose pairs
        for g in (3, 2):
            blocks = list(range(4 * g + 3, 4 * g - 1, -1))
            for b in blocks:
                cast_block(b)
            for i, b in enumerate(blocks):
                transpose_block(b)
                fill(2)
                if i == 1:
                    fill(8)
                elif i == 2:
                    # bridge the wait for the group's last block (qk needs it)
                    fill(6)
            qk_proj(g)
            for j in blocks:
                stripe(j)
            # post-stripe fill bridges: the PE finishes each group's burst
            # before the next group's x blocks land
            fill(10 if g == 3 else 8)
        vq = list(range(15, -1, -1))  # v-proj emission queue
        for g in (1, 0):
            blocks = list(range(4 * g + 3, 4 * g - 1, -1))
            for b in blocks:
                cast_block(b)
            for i, b in enumerate(blocks):
                transpose_block(b)
                if i == 1:
                    v_proj(vq.pop(0))
                    v_proj(vq.pop(0))
            qk_proj(g)
            for j in blocks:
                stripe(j)
            v_proj(vq.pop(0))
            v_proj(vq.pop(0))
        while vq:
            v_proj(vq.pop(0))
        # A@V descending: big blocks first, epilogues hide under the next
        # block's matmuls, and the tail ends on the smallest ones
        for i in range(NSB - 1, -1, -1):
            av(i)

    nc.compile()
    return nc


def _get_nc():
    if "nc" not in _CACHE:
        _CACHE["nc"] = _build()
    return _CACHE["nc"]


def kernel(input, Wq, bq, Wk, bk, Wv, bv):
    from concourse.bass_utils import run_bass_kernel_spmd

    nc = _get_nc()
    x = np.ascontiguousarray(np.asarray(input, dtype=np.float32))
    ident = np.eye(P, dtype=np.float32)
    # transposed causal mask for S^T diagonal blocks: valid iff k <= q
    maskT = np.where(
        np.arange(P)[:, None] <= np.arange(P)[None, :], 0.0, NEG
    ).astype(np.float32)
    bv_np = np.asarray(bv, dtype=np.float32)
    common = {
        "wq": np.ascontiguousarray(np.asarray(Wq, dtype=np.float32)),
        "bq": np.ascontiguousarray(np.asarray(bq, dtype=np.float32)),
        "wk": np.ascontiguousarray(np.asarray(Wk, dtype=np.float32)),
        "bk": np.ascontiguousarray(np.asarray(bk, dtype=np.float32)),
        "wv": np.ascontiguousarray(np.asarray(Wv, dtype=np.float32)),
        "bvbc": np.ascontiguousarray(np.tile(bv_np[None, :], (P, 1))),
        "ident": ident,
        "maskt": maskT,
    }
    in_maps = [dict(common, x=np.ascontiguousarray(x[c])) for c in range(B)]
    res = run_bass_kernel_spmd(nc, in_maps, core_ids=list(range(B)))
    return np.stack([res.results[c]["o"] for c in range(B)], axis=0)
